# revision 28
# baseline (speedup 1.0000x reference)
"""KPConv aggregate layer on 8 trn2 NeuronCores.

Math (per batch b):
    sq_d[n,k]  = ||p[n] - kp[k]||^2
    aw[n,k]    = relu(1 - sqrt(sq_d)/KP_EXTENT)
    wf[k,c]    = sum_n aw[n,k] * x[c,n]
    out[o]     = sum_{k,c} wf[k,c] * W[k,c,o]

Sharding: data-parallel over B=8 across the 8 cores (batch b -> core b).

aw has a radius cutoff, so only columns n with min_k ||p[n]-kp[k]|| <
KP_EXTENT contribute (~17.5% of N on N(0,1) points).  The host gathers
the active columns of x and their point coords and ships only those —
everything else is exact zeros.  The dominant cost is the axon tunnel
(~75 MB/s aggregate, ~100 ms RTT), so x is shipped as int8 with a
per-column max scale; the device converts int8->fp16 and the dequant
scale is folded into aw (recomputed on device from the gathered
coords, then multiplied by the shipped scale vector).  The device
kernel PE-transposes the x tiles and accumulates wf with 15-wide
stationary matmuls into PSUM, then applies the tiny [15,128,128] GEMM.

The PJRT executable (jit of shard_map over the 8 cores) is built once
and cached, replicated constants stay device-resident, per-batch shards
are uploaded asynchronously so packing overlaps the transfer, and the
result fetch is requested before blocking so its RTT hides under the
input transfer.  If an input activates more columns than the compiled
capacity CH*128, a numpy fallback computes the exact result.
"""

import numpy as np
from contextlib import ExitStack

import jax
from jax.sharding import Mesh, PartitionSpec, NamedSharding

import concourse.bass as bass
import concourse.mybir as mybir
import concourse.tile as tile
from concourse import bacc
from concourse.bass2jax import (
    _bass_exec_p,
    install_neuronx_cc_hook,
    partition_id_tensor,
)

try:
    from jax.experimental.shard_map import shard_map
except ImportError:
    from jax import shard_map

B, N, C, K = 8, 65536, 128, 15
KP_EXTENT = 1.0 * 1.2 / 2.5  # 0.48
CH = 96               # compiled capacity: chunks of 128 gathered columns
L = CH * 128          # 12288 gathered columns per core
XT = 2048             # x DMA tile free size
# block widths: full 2048-tiles then a 512-multiple remainder
_BLOCKS = []
_off = 0
while _off < L:
    _w = min(XT, L - _off)
    _BLOCKS.append((_off, _w))
    _off += _w

f32 = mybir.dt.float32
f16 = mybir.dt.float16


def _ap3(t, off_elems, d1, d2):
    """Build a 3-D access pattern [pdim, d1, d2] over tile ap `t`."""
    return bass.AP(t.tensor, t.offset + off_elems, [t.ap[0][:], list(d1), list(d2)])


def build_nc():
    nc = bacc.Bacc("TRN2", target_bir_lowering=False, debug=False, num_devices=B)

    i8 = mybir.dt.int8
    xq_d = nc.dram_tensor("xq", [C, L], i8, kind="ExternalInput")
    sg_d = nc.dram_tensor("sg", [128, CH], f16, kind="ExternalInput")
    pg_d = nc.dram_tensor("pg", [128, 3 * CH], f16, kind="ExternalInput")
    kb3_d = nc.dram_tensor("kb3", [128, 3 * K], f16, kind="ExternalInput")
    wsb_d = nc.dram_tensor("wsb", [C, K * 128], f16, kind="ExternalInput")
    eye16_d = nc.dram_tensor("eye16", [128, 128], f16, kind="ExternalInput")
    out_d = nc.dram_tensor("out", [1, 128], f32, kind="ExternalOutput")

    with tile.TileContext(nc) as tc, ExitStack() as ctx:
        consts = ctx.enter_context(tc.tile_pool(name="consts", bufs=1))
        tmp = ctx.enter_context(tc.tile_pool(name="tmp", bufs=3))
        xpool = ctx.enter_context(tc.tile_pool(name="xpool", bufs=3))
        xspool = ctx.enter_context(tc.tile_pool(name="xspool", bufs=6))
        ps_x = ctx.enter_context(tc.tile_pool(name="ps_x", bufs=4, space="PSUM"))
        ps_sm = ctx.enter_context(tc.tile_pool(name="ps_sm", bufs=2, space="PSUM"))
        ps_wf = ctx.enter_context(tc.tile_pool(name="ps_wf", bufs=1, space="PSUM"))
        fin = ctx.enter_context(tc.tile_pool(name="fin", bufs=1))

        eye16 = consts.tile([128, 128], f16)
        nc.sync.dma_start(eye16, eye16_d.ap())
        wsb = consts.tile([C, K * 128], f16)
        nc.sync.dma_start(wsb, wsb_d.ap())
        pg = consts.tile([128, 3 * CH], f16)
        nc.sync.dma_start(pg, pg_d.ap())
        sg = consts.tile([128, CH], f16)
        nc.sync.dma_start(sg, sg_d.ap())
        kb3 = consts.tile([128, 3 * K], f16)
        nc.sync.dma_start(kb3, kb3_d.ap())

        # aw[j, ch*K+k] = relu(1 - |p_active[ch*128+j] - kp[k]| / KP_EXTENT)
        awb = consts.tile([128, CH * K], f16)
        acc = None
        for d in range(3):
            dx = tmp.tile([128, CH * K], f16, tag="dx", name=f"dx{d}")
            dx3 = _ap3(dx, 0, [K, CH], [1, K])
            pb = _ap3(pg, d * CH, [1, CH], [0, K])
            kb = _ap3(kb3, d * K, [0, CH], [1, K])
            nc.vector.tensor_tensor(dx3, pb, kb, op=mybir.AluOpType.subtract)
            sx = tmp.tile([128, CH * K], f16, tag="sx", name=f"sx{d}")
            nc.vector.tensor_tensor(sx, dx, dx, op=mybir.AluOpType.mult)
            if acc is None:
                acc = sx
            else:
                a2 = tmp.tile([128, CH * K], f16, tag="acc", name=f"acc{d}")
                nc.vector.tensor_tensor(a2, acc, sx, op=mybir.AluOpType.add)
                acc = a2
        rt = tmp.tile([128, CH * K], f16, tag="rt")
        nc.scalar.sqrt(rt, acc)
        awr = tmp.tile([128, CH * K], f16, tag="awr")
        nc.scalar.activation(
            awr, rt, mybir.ActivationFunctionType.Relu,
            bias=1.0, scale=-1.0 / KP_EXTENT)
        # fold the int8 per-column dequant scale into aw
        sgb = _ap3(sg, 0, [1, CH], [0, K])
        awr3 = _ap3(awr, 0, [K, CH], [1, K])
        awb3 = _ap3(awb, 0, [K, CH], [1, K])
        nc.vector.tensor_tensor(awb3, awr3, sgb, op=mybir.AluOpType.mult)

        # wf[k,c] accumulated over all CH chunks of gathered columns
        wf = ps_wf.tile([K, 128], f32)
        for j, (off, w) in enumerate(_BLOCKS):
            xq = xpool.tile([128, w], i8, tag="xq", name=f"xq{j}")
            nc.sync.dma_start(xq, xq_d.ap()[:, off:off + w])
            # int8 -> f16 numeric conversion (dequant scale lives in awb)
            xt = xpool.tile([128, w], f16, tag="xt", name=f"xt{j}")
            nc.vector.tensor_copy(xt, xq)
            h0 = 0
            while h0 < w:
                hw = min(1024, w - h0)
                ps = ps_x.tile([128, hw], f16, tag="psx", name=f"psx{j}{h0}")
                for u in range(hw // 128):
                    nc.tensor.transpose(
                        ps[:, 128 * u:128 * (u + 1)],
                        xt[:, h0 + 128 * u:h0 + 128 * (u + 1)],
                        eye16)
                xs = xspool.tile([128, hw], f16, tag="xs", name=f"xs{j}{h0}")
                nc.vector.tensor_copy(xs, ps)
                for u in range(hw // 128):
                    ch = (off + h0) // 128 + u
                    nc.tensor.matmul(
                        wf, awb[:, K * ch:K * (ch + 1)],
                        xs[:, 128 * u:128 * (u + 1)],
                        start=(ch == 0), stop=(ch == CH - 1),
                        skip_group_check=True)
                h0 += hw

        # stage 2: out[o] = sum_k wf[k,:] @ W[k]
        wf_sb = fin.tile([K, 128], f16)
        nc.vector.tensor_copy(wf_sb, wf)
        wft_ps = ps_sm.tile([128, K], f16, tag="pt")
        nc.tensor.transpose(wft_ps, wf_sb, eye16[:K, :K])
        wft = fin.tile([128, K], f16)
        nc.vector.tensor_copy(wft, wft_ps)
        o_ps = ps_sm.tile([1, 128], f32, tag="pt")
        for k in range(K):
            nc.tensor.matmul(
                o_ps, wft[:, k:k + 1], wsb[:, 128 * k:128 * (k + 1)],
                start=(k == 0), stop=(k == K - 1), skip_group_check=True)
        o_sb = fin.tile([1, 128], f32)
        nc.vector.tensor_copy(o_sb, o_ps)
        nc.sync.dma_start(out_d.ap(), o_sb)

    nc.compile()
    return nc


def _host_aw(p, kp):
    """aw[b,n,k] = relu(1 - |p[b,n]-kp[k]|/KP_EXTENT), f32, exact."""
    d2 = ((p * p).sum(-1)[:, :, None] + (kp * kp).sum(-1)[None, None, :]
          - 2.0 * (p @ kp.T))
    aw = 1.0 - np.sqrt(np.maximum(d2, 0.0)) * np.float32(1.0 / KP_EXTENT)
    return np.maximum(aw, 0.0, out=aw)


def _active_mask(p, kp):
    """n is active iff min_k ||p[n]-kp[k]||^2 < KP_EXTENT^2 (no sqrt)."""
    d2 = ((p * p).sum(-1)[:, :, None] + (kp * kp).sum(-1)[None, None, :]
          - 2.0 * (p @ kp.T))
    return d2.min(axis=2) < np.float32(KP_EXTENT * KP_EXTENT)


def _pack_batch(pb, xb, idx):
    """Pack one batch's active columns -> (xq, sg, pg) arrays [128, ...]."""
    m = idx.size
    xq = np.zeros((128, L), np.int8)
    s = np.zeros(0, np.float32)
    if m:
        xa = xb[:, idx]                                   # [128, m] f32
        s = np.abs(xa).max(axis=0) * np.float32(1 / 127)  # per-column scale
        np.maximum(s, np.float32(1e-30), out=s)
        xq[:, :m] = np.rint(xa / s).astype(np.int8)
    # sg[j, ch] = s of point ch*128+j (0 for padding -> aw*s = 0)
    s_pad = np.zeros(L, np.float32)
    s_pad[:m] = s
    sg = np.ascontiguousarray(s_pad.reshape(CH, 128).T).astype(np.float16)
    p_pad = np.zeros((L, 3), np.float32)
    p_pad[:m] = pb[idx]
    # pg[j, d*CH+ch] = p_active[ch*128+j, d]
    pg = np.ascontiguousarray(
        p_pad.reshape(CH, 128, 3).transpose(1, 2, 0).reshape(128, 3 * CH)
    ).astype(np.float16)
    return {"xq": xq, "sg": sg, "pg": pg}


def pack_inputs(p, x, weights, kernel_points):
    """Gather active columns; build concat-ready [B*128, ...] arrays.

    Returns None if any batch activates more than L columns (caller
    falls back to the exact numpy path)."""
    p = np.asarray(p, np.float32)
    x = np.asarray(x, np.float32)
    kp = np.asarray(kernel_points, np.float32)

    act = _active_mask(p, kp)
    out = {"xq": np.empty((B * 128, L), np.int8),
           "sg": np.empty((B * 128, CH), np.float16),
           "pg": np.empty((B * 128, 3 * CH), np.float16)}
    for b in range(B):
        idx = np.flatnonzero(act[b])
        if idx.size > L:
            return None
        for name, arr in _pack_batch(p[b], x[b], idx).items():
            out[name][b * 128:(b + 1) * 128] = arr
    return out


def pack_consts(weights, kernel_points):
    w = np.asarray(weights, np.float32)
    kp = np.asarray(kernel_points, np.float32)
    wsb = np.ascontiguousarray(
        w.transpose(1, 0, 2).reshape(C, K * 128)).astype(np.float16)
    eye16 = np.eye(128, dtype=np.float16)
    # kb3[j, d*K+k] = kp[k, d], rows replicated
    kb3 = np.ascontiguousarray(
        np.broadcast_to(kp.T.reshape(1, 3 * K), (128, 3 * K))).astype(np.float16)

    def rep(a):
        return np.ascontiguousarray(
            np.broadcast_to(a[None], (B, *a.shape))).reshape(B * a.shape[0],
                                                             *a.shape[1:])
    return {"wsb": rep(wsb), "eye16": rep(eye16), "kb3": rep(kb3)}


class Runner:
    """Persistent jit of shard_map(bass_exec) over the 8 cores."""

    def __init__(self):
        install_neuronx_cc_hook()
        self.nc = nc = build_nc()
        pname = nc.partition_id_tensor.name if nc.partition_id_tensor else None
        in_names, out_names, out_avals = [], [], []
        for alloc in nc.m.functions[0].allocations:
            if not isinstance(alloc, mybir.MemoryLocationSet):
                continue
            name = alloc.memorylocations[0].name
            if alloc.kind == "ExternalInput":
                if name != pname:
                    in_names.append(name)
            elif alloc.kind == "ExternalOutput":
                out_names.append(name)
                out_avals.append(jax.core.ShapedArray(
                    tuple(alloc.tensor_shape), mybir.dt.np(alloc.dtype)))
        self.in_names, self.out_names, self.out_avals = in_names, out_names, out_avals
        all_in = list(in_names) + list(out_names)
        if pname is not None:
            all_in.append(pname)
        n_params, n_outs = len(in_names), len(out_names)
        donate = tuple(range(n_params, n_params + n_outs))

        def _body(*args):
            operands = list(args)
            if pname is not None:
                operands.append(partition_id_tensor())
            return tuple(_bass_exec_p.bind(
                *operands,
                out_avals=tuple(out_avals),
                in_names=tuple(all_in),
                out_names=tuple(out_names),
                lowering_input_output_aliases=(),
                sim_require_finite=True,
                sim_require_nnan=True,
                nc=nc,
            ))

        devices = jax.devices()[:B]
        self.mesh = Mesh(np.asarray(devices), ("core",))
        self.sharding = NamedSharding(self.mesh, PartitionSpec("core"))
        in_specs = (PartitionSpec("core"),) * (n_params + n_outs)
        out_specs = (PartitionSpec("core"),) * n_outs
        self.fn = jax.jit(
            shard_map(_body, mesh=self.mesh, in_specs=in_specs,
                      out_specs=out_specs, check_rep=False),
            donate_argnums=donate, keep_unused=True)
        self._const_key = None
        self._const_dev = None

    def put_consts(self, weights, kernel_points):
        """Device-resident replicated constants, re-uploaded only when
        the weights / kernel points actually change."""
        w = np.asarray(weights)
        kp = np.asarray(kernel_points)
        key = hash((w.tobytes(), kp.tobytes()))
        if key != self._const_key:
            consts = pack_consts(w, kp)
            self._const_dev = {
                k: jax.device_put(v, self.sharding) for k, v in consts.items()}
            self._const_key = key
        return self._const_dev

    def run(self, packed, const_dev):
        args = []
        for name in self.in_names:
            args.append(packed[name] if name in packed else const_dev[name])
        zeros = [np.zeros((B * a.shape[0], *a.shape[1:]), a.dtype)
                 for a in self.out_avals]
        outs = self.fn(*args, *zeros)
        # request the (tiny) result right away so the D2H round trip
        # overlaps the input transfer + execution instead of following it
        outs[0].copy_to_host_async()
        out = np.asarray(outs[0]).reshape(B, *self.out_avals[0].shape)
        return out.reshape(B, -1)


_RUNNER = None


def _get_runner():
    global _RUNNER
    if _RUNNER is None:
        _RUNNER = Runner()
    return _RUNNER


def _numpy_fallback(p, x, weights, kernel_points):
    aw = _host_aw(np.asarray(p, np.float32), np.asarray(kernel_points, np.float32))
    wf = np.einsum('bnk,bcn->bkc', aw, np.asarray(x, np.float32))
    return np.einsum('bkc,kco->bo', wf, np.asarray(weights, np.float32))


_GSHAPES = {"xq": (B * 128, L), "sg": (B * 128, CH),
            "pg": (B * 128, 3 * CH)}


def kernel(p, x, weights, kernel_points):
    p_ = np.asarray(p, np.float32)
    x_ = np.asarray(x, np.float32)
    kp_ = np.asarray(kernel_points, np.float32)

    r = _get_runner()
    const_dev = r.put_consts(weights, kernel_points)
    act = _active_mask(p_, kp_)
    idxs = [np.flatnonzero(act[b]) for b in range(B)]
    if max(i.size for i in idxs) > L:  # beyond compiled capacity
        return _numpy_fallback(p, x, weights, kernel_points).astype(np.float32)

    # pack batch-by-batch, shipping each core's shard asynchronously so the
    # host packing overlaps the (slow) host->device transfer
    devices = list(r.mesh.devices.flat)
    shards = {name: [] for name in _GSHAPES}
    for b in range(B):
        batch = _pack_batch(p_[b], x_[b], idxs[b])
        for name in shards:
            shards[name].append(jax.device_put(batch[name], devices[b]))
    packed = {name: jax.make_array_from_single_device_arrays(
        _GSHAPES[name], r.sharding, shards[name]) for name in shards}
    return r.run(packed, const_dev).astype(np.float32)


# revision 29
# speedup vs baseline: 1.0549x; 1.0549x over previous
"""KPConv aggregate layer on 8 trn2 NeuronCores.

Math (per batch b):
    sq_d[n,k]  = ||p[n] - kp[k]||^2
    aw[n,k]    = relu(1 - sqrt(sq_d)/KP_EXTENT)
    wf[k,c]    = sum_n aw[n,k] * x[c,n]
    out[o]     = sum_{k,c} wf[k,c] * W[k,c,o]

Sharding: data-parallel over B=8 across the 8 cores (batch b -> core b).

aw has a radius cutoff, so only columns n with min_k ||p[n]-kp[k]|| <
KP_EXTENT contribute (~17.5% of N on N(0,1) points).  The host gathers
the active columns of x and their point coords and ships only those —
everything else is exact zeros.  The dominant cost is the axon tunnel
(~75 MB/s aggregate, ~100 ms RTT), so x is shipped as int8 with a
per-column max scale; the device converts int8->fp16 and the dequant
scale is folded into aw (recomputed on device from the gathered
coords, then multiplied by the shipped scale vector).  The device
kernel PE-transposes the x tiles and accumulates wf with 15-wide
stationary matmuls into PSUM, then applies the tiny [15,128,128] GEMM.

The PJRT executable (jit of shard_map over the 8 cores) is built once
and cached, replicated constants stay device-resident, per-batch shards
are uploaded asynchronously so packing overlaps the transfer, and the
result fetch is requested before blocking so its RTT hides under the
input transfer.  If an input activates more columns than the compiled
capacity CH*128, a numpy fallback computes the exact result.
"""

import numpy as np
from contextlib import ExitStack

import jax
from jax.sharding import Mesh, PartitionSpec, NamedSharding

import concourse.bass as bass
import concourse.mybir as mybir
import concourse.tile as tile
from concourse import bacc
from concourse.bass2jax import (
    _bass_exec_p,
    install_neuronx_cc_hook,
    partition_id_tensor,
)

try:
    from jax.experimental.shard_map import shard_map
except ImportError:
    from jax import shard_map

B, N, C, K = 8, 65536, 128, 15
KP_EXTENT = 1.0 * 1.2 / 2.5  # 0.48
CH = 96               # compiled capacity: chunks of 128 gathered columns
L = CH * 128          # 12288 gathered columns per core
XT = 2048             # x DMA tile free size
# block widths: full 2048-tiles then a 512-multiple remainder
_BLOCKS = []
_off = 0
while _off < L:
    _w = min(XT, L - _off)
    _BLOCKS.append((_off, _w))
    _off += _w

f32 = mybir.dt.float32
f16 = mybir.dt.float16


def _ap3(t, off_elems, d1, d2):
    """Build a 3-D access pattern [pdim, d1, d2] over tile ap `t`."""
    return bass.AP(t.tensor, t.offset + off_elems, [t.ap[0][:], list(d1), list(d2)])


def build_nc():
    nc = bacc.Bacc("TRN2", target_bir_lowering=False, debug=False, num_devices=B)

    i8 = mybir.dt.int8
    xq_d = nc.dram_tensor("xq", [C, L], i8, kind="ExternalInput")
    sg_d = nc.dram_tensor("sg", [128, CH], f16, kind="ExternalInput")
    pg_d = nc.dram_tensor("pg", [128, 3 * CH], f16, kind="ExternalInput")
    kb3_d = nc.dram_tensor("kb3", [128, 3 * K], f16, kind="ExternalInput")
    wsb_d = nc.dram_tensor("wsb", [C, K * 128], f16, kind="ExternalInput")
    eye16_d = nc.dram_tensor("eye16", [128, 128], f16, kind="ExternalInput")
    out_d = nc.dram_tensor("out", [1, 128], f32, kind="ExternalOutput")

    with tile.TileContext(nc) as tc, ExitStack() as ctx:
        consts = ctx.enter_context(tc.tile_pool(name="consts", bufs=1))
        tmp = ctx.enter_context(tc.tile_pool(name="tmp", bufs=3))
        xpool = ctx.enter_context(tc.tile_pool(name="xpool", bufs=3))
        xspool = ctx.enter_context(tc.tile_pool(name="xspool", bufs=6))
        ps_x = ctx.enter_context(tc.tile_pool(name="ps_x", bufs=4, space="PSUM"))
        ps_sm = ctx.enter_context(tc.tile_pool(name="ps_sm", bufs=2, space="PSUM"))
        ps_wf = ctx.enter_context(tc.tile_pool(name="ps_wf", bufs=1, space="PSUM"))
        fin = ctx.enter_context(tc.tile_pool(name="fin", bufs=1))

        eye16 = consts.tile([128, 128], f16)
        nc.sync.dma_start(eye16, eye16_d.ap())
        wsb = consts.tile([C, K * 128], f16)
        nc.sync.dma_start(wsb, wsb_d.ap())
        pg = consts.tile([128, 3 * CH], f16)
        nc.sync.dma_start(pg, pg_d.ap())
        sg = consts.tile([128, CH], f16)
        nc.sync.dma_start(sg, sg_d.ap())
        kb3 = consts.tile([128, 3 * K], f16)
        nc.sync.dma_start(kb3, kb3_d.ap())

        # aw[j, ch*K+k] = relu(1 - |p_active[ch*128+j] - kp[k]| / KP_EXTENT)
        awb = consts.tile([128, CH * K], f16)
        acc = None
        for d in range(3):
            dx = tmp.tile([128, CH * K], f16, tag="dx", name=f"dx{d}")
            dx3 = _ap3(dx, 0, [K, CH], [1, K])
            pb = _ap3(pg, d * CH, [1, CH], [0, K])
            kb = _ap3(kb3, d * K, [0, CH], [1, K])
            nc.vector.tensor_tensor(dx3, pb, kb, op=mybir.AluOpType.subtract)
            sx = tmp.tile([128, CH * K], f16, tag="sx", name=f"sx{d}")
            nc.vector.tensor_tensor(sx, dx, dx, op=mybir.AluOpType.mult)
            if acc is None:
                acc = sx
            else:
                a2 = tmp.tile([128, CH * K], f16, tag="acc", name=f"acc{d}")
                nc.vector.tensor_tensor(a2, acc, sx, op=mybir.AluOpType.add)
                acc = a2
        rt = tmp.tile([128, CH * K], f16, tag="rt")
        nc.scalar.sqrt(rt, acc)
        awr = tmp.tile([128, CH * K], f16, tag="awr")
        nc.scalar.activation(
            awr, rt, mybir.ActivationFunctionType.Relu,
            bias=1.0, scale=-1.0 / KP_EXTENT)
        # fold the int8 per-column dequant scale into aw
        sgb = _ap3(sg, 0, [1, CH], [0, K])
        awr3 = _ap3(awr, 0, [K, CH], [1, K])
        awb3 = _ap3(awb, 0, [K, CH], [1, K])
        nc.vector.tensor_tensor(awb3, awr3, sgb, op=mybir.AluOpType.mult)

        # wf[k,c] accumulated over all CH chunks of gathered columns
        wf = ps_wf.tile([K, 128], f32)
        for j, (off, w) in enumerate(_BLOCKS):
            xq = xpool.tile([128, w], i8, tag="xq", name=f"xq{j}")
            nc.sync.dma_start(xq, xq_d.ap()[:, off:off + w])
            # int8 -> f16 numeric conversion (dequant scale lives in awb)
            xt = xpool.tile([128, w], f16, tag="xt", name=f"xt{j}")
            nc.vector.tensor_copy(xt, xq)
            h0 = 0
            while h0 < w:
                hw = min(1024, w - h0)
                ps = ps_x.tile([128, hw], f16, tag="psx", name=f"psx{j}{h0}")
                for u in range(hw // 128):
                    nc.tensor.transpose(
                        ps[:, 128 * u:128 * (u + 1)],
                        xt[:, h0 + 128 * u:h0 + 128 * (u + 1)],
                        eye16)
                xs = xspool.tile([128, hw], f16, tag="xs", name=f"xs{j}{h0}")
                nc.vector.tensor_copy(xs, ps)
                for u in range(hw // 128):
                    ch = (off + h0) // 128 + u
                    nc.tensor.matmul(
                        wf, awb[:, K * ch:K * (ch + 1)],
                        xs[:, 128 * u:128 * (u + 1)],
                        start=(ch == 0), stop=(ch == CH - 1),
                        skip_group_check=True)
                h0 += hw

        # stage 2: out[o] = sum_k wf[k,:] @ W[k]
        wf_sb = fin.tile([K, 128], f16)
        nc.vector.tensor_copy(wf_sb, wf)
        wft_ps = ps_sm.tile([128, K], f16, tag="pt")
        nc.tensor.transpose(wft_ps, wf_sb, eye16[:K, :K])
        wft = fin.tile([128, K], f16)
        nc.vector.tensor_copy(wft, wft_ps)
        o_ps = ps_sm.tile([1, 128], f32, tag="pt")
        for k in range(K):
            nc.tensor.matmul(
                o_ps, wft[:, k:k + 1], wsb[:, 128 * k:128 * (k + 1)],
                start=(k == 0), stop=(k == K - 1), skip_group_check=True)
        o_sb = fin.tile([1, 128], f32)
        nc.vector.tensor_copy(o_sb, o_ps)
        nc.sync.dma_start(out_d.ap(), o_sb)

    nc.compile()
    return nc


def _host_aw(p, kp):
    """aw[b,n,k] = relu(1 - |p[b,n]-kp[k]|/KP_EXTENT), f32, exact."""
    d2 = ((p * p).sum(-1)[:, :, None] + (kp * kp).sum(-1)[None, None, :]
          - 2.0 * (p @ kp.T))
    aw = 1.0 - np.sqrt(np.maximum(d2, 0.0)) * np.float32(1.0 / KP_EXTENT)
    return np.maximum(aw, 0.0, out=aw)


def _active_mask(p, kp):
    """n is active iff min_k ||p[n]-kp[k]||^2 < KP_EXTENT^2 (no sqrt)."""
    d2 = ((p * p).sum(-1)[:, :, None] + (kp * kp).sum(-1)[None, None, :]
          - 2.0 * (p @ kp.T))
    return d2.min(axis=2) < np.float32(KP_EXTENT * KP_EXTENT)


def _pack_batch(pb, xb, idx):
    """Pack one batch's active columns -> (xq, sg, pg) arrays [128, ...]."""
    m = idx.size
    xq = np.zeros((128, L), np.int8)
    s = np.zeros(0, np.float32)
    if m:
        xa = xb[:, idx]                                   # [128, m] f32
        s = np.abs(xa).max(axis=0) * np.float32(1 / 127)  # per-column scale
        np.maximum(s, np.float32(1e-30), out=s)
        xq[:, :m] = np.rint(xa / s).astype(np.int8)
    # sg[j, ch] = s of point ch*128+j (0 for padding -> aw*s = 0)
    s_pad = np.zeros(L, np.float32)
    s_pad[:m] = s
    sg = np.ascontiguousarray(s_pad.reshape(CH, 128).T).astype(np.float16)
    p_pad = np.zeros((L, 3), np.float32)
    p_pad[:m] = pb[idx]
    # pg[j, d*CH+ch] = p_active[ch*128+j, d]
    pg = np.ascontiguousarray(
        p_pad.reshape(CH, 128, 3).transpose(1, 2, 0).reshape(128, 3 * CH)
    ).astype(np.float16)
    return {"xq": xq, "sg": sg, "pg": pg}


def pack_inputs(p, x, weights, kernel_points):
    """Gather active columns; build concat-ready [B*128, ...] arrays.

    Returns None if any batch activates more than L columns (caller
    falls back to the exact numpy path)."""
    p = np.asarray(p, np.float32)
    x = np.asarray(x, np.float32)
    kp = np.asarray(kernel_points, np.float32)

    act = _active_mask(p, kp)
    out = {"xq": np.empty((B * 128, L), np.int8),
           "sg": np.empty((B * 128, CH), np.float16),
           "pg": np.empty((B * 128, 3 * CH), np.float16)}
    for b in range(B):
        idx = np.flatnonzero(act[b])
        if idx.size > L:
            return None
        for name, arr in _pack_batch(p[b], x[b], idx).items():
            out[name][b * 128:(b + 1) * 128] = arr
    return out


def pack_consts(weights, kernel_points):
    w = np.asarray(weights, np.float32)
    kp = np.asarray(kernel_points, np.float32)
    wsb = np.ascontiguousarray(
        w.transpose(1, 0, 2).reshape(C, K * 128)).astype(np.float16)
    eye16 = np.eye(128, dtype=np.float16)
    # kb3[j, d*K+k] = kp[k, d], rows replicated
    kb3 = np.ascontiguousarray(
        np.broadcast_to(kp.T.reshape(1, 3 * K), (128, 3 * K))).astype(np.float16)

    def rep(a):
        return np.ascontiguousarray(
            np.broadcast_to(a[None], (B, *a.shape))).reshape(B * a.shape[0],
                                                             *a.shape[1:])
    return {"wsb": rep(wsb), "eye16": rep(eye16), "kb3": rep(kb3)}


class Runner:
    """Persistent jit of shard_map(bass_exec) over the 8 cores."""

    def __init__(self):
        install_neuronx_cc_hook()
        self.nc = nc = build_nc()
        pname = nc.partition_id_tensor.name if nc.partition_id_tensor else None
        in_names, out_names, out_avals = [], [], []
        for alloc in nc.m.functions[0].allocations:
            if not isinstance(alloc, mybir.MemoryLocationSet):
                continue
            name = alloc.memorylocations[0].name
            if alloc.kind == "ExternalInput":
                if name != pname:
                    in_names.append(name)
            elif alloc.kind == "ExternalOutput":
                out_names.append(name)
                out_avals.append(jax.core.ShapedArray(
                    tuple(alloc.tensor_shape), mybir.dt.np(alloc.dtype)))
        self.in_names, self.out_names, self.out_avals = in_names, out_names, out_avals
        all_in = list(in_names) + list(out_names)
        if pname is not None:
            all_in.append(pname)
        n_params, n_outs = len(in_names), len(out_names)
        donate = tuple(range(n_params, n_params + n_outs))

        def _body(*args):
            operands = list(args)
            if pname is not None:
                operands.append(partition_id_tensor())
            return tuple(_bass_exec_p.bind(
                *operands,
                out_avals=tuple(out_avals),
                in_names=tuple(all_in),
                out_names=tuple(out_names),
                lowering_input_output_aliases=(),
                sim_require_finite=True,
                sim_require_nnan=True,
                nc=nc,
            ))

        devices = jax.devices()[:B]
        self.mesh = Mesh(np.asarray(devices), ("core",))
        self.sharding = NamedSharding(self.mesh, PartitionSpec("core"))
        in_specs = (PartitionSpec("core"),) * (n_params + n_outs)
        out_specs = (PartitionSpec("core"),) * n_outs
        self.fn = jax.jit(
            shard_map(_body, mesh=self.mesh, in_specs=in_specs,
                      out_specs=out_specs, check_rep=False),
            donate_argnums=donate, keep_unused=True)
        self._const_key = None
        self._const_dev = None

    def put_consts(self, weights, kernel_points):
        """Device-resident replicated constants, re-uploaded only when
        the weights / kernel points actually change."""
        ids = (id(weights), id(kernel_points))
        if ids == getattr(self, "_const_ids", None):  # same arrays, fast path
            return self._const_dev
        w = np.asarray(weights)
        kp = np.asarray(kernel_points)
        key = hash((w.tobytes(), kp.tobytes()))
        if key != self._const_key:
            consts = pack_consts(w, kp)
            self._const_dev = {
                k: jax.device_put(v, self.sharding) for k, v in consts.items()}
            self._const_key = key
        self._const_ids = ids
        return self._const_dev

    def run(self, packed, const_dev):
        args = []
        for name in self.in_names:
            args.append(packed[name] if name in packed else const_dev[name])
        zeros = [np.zeros((B * a.shape[0], *a.shape[1:]), a.dtype)
                 for a in self.out_avals]
        outs = self.fn(*args, *zeros)
        # request the (tiny) result right away so the D2H round trip
        # overlaps the input transfer + execution instead of following it
        outs[0].copy_to_host_async()
        out = np.asarray(outs[0]).reshape(B, *self.out_avals[0].shape)
        return out.reshape(B, -1)


_RUNNER = None


def _get_runner():
    global _RUNNER
    if _RUNNER is None:
        _RUNNER = Runner()
    return _RUNNER


def _numpy_fallback(p, x, weights, kernel_points):
    aw = _host_aw(np.asarray(p, np.float32), np.asarray(kernel_points, np.float32))
    wf = np.einsum('bnk,bcn->bkc', aw, np.asarray(x, np.float32))
    return np.einsum('bkc,kco->bo', wf, np.asarray(weights, np.float32))


_GSHAPES = {"xq": (B * 128, L), "sg": (B * 128, CH),
            "pg": (B * 128, 3 * CH)}


def kernel(p, x, weights, kernel_points):
    p_ = np.asarray(p, np.float32)
    x_ = np.asarray(x, np.float32)
    kp_ = np.asarray(kernel_points, np.float32)

    r = _get_runner()
    const_dev = r.put_consts(weights, kernel_points)
    act = _active_mask(p_, kp_)
    idxs = [np.flatnonzero(act[b]) for b in range(B)]
    if max(i.size for i in idxs) > L:  # beyond compiled capacity
        return _numpy_fallback(p, x, weights, kernel_points).astype(np.float32)

    # pack batch-by-batch, shipping each core's shard asynchronously so the
    # host packing overlaps the (slow) host->device transfer
    devices = list(r.mesh.devices.flat)
    shards = {name: [] for name in _GSHAPES}
    for b in range(B):
        batch = _pack_batch(p_[b], x_[b], idxs[b])
        for name in shards:
            shards[name].append(jax.device_put(batch[name], devices[b]))
    packed = {name: jax.make_array_from_single_device_arrays(
        _GSHAPES[name], r.sharding, shards[name]) for name in shards}
    return r.run(packed, const_dev).astype(np.float32)


# revision 30
# speedup vs baseline: 1.0937x; 1.0368x over previous
"""KPConv aggregate layer on 8 trn2 NeuronCores.

Math (per batch b):
    sq_d[n,k]  = ||p[n] - kp[k]||^2
    aw[n,k]    = relu(1 - sqrt(sq_d)/KP_EXTENT)
    wf[k,c]    = sum_n aw[n,k] * x[c,n]
    out[o]     = sum_{k,c} wf[k,c] * W[k,c,o]

Sharding: data-parallel over B=8 across the 8 cores (batch b -> core b).

aw has a radius cutoff, so only columns n with min_k ||p[n]-kp[k]|| <
KP_EXTENT contribute (~17.5% of N on N(0,1) points).  The host gathers
the active columns of x and their point coords and ships only those —
everything else is exact zeros.  The dominant cost is the axon tunnel
(~75 MB/s aggregate, ~100 ms RTT), so x is shipped as int8 with a
per-column max scale; the device converts int8->fp16 and the dequant
scale is folded into aw (recomputed on device from the gathered
coords, then multiplied by the shipped scale vector).  The device
kernel PE-transposes the x tiles and accumulates wf with 15-wide
stationary matmuls into PSUM, then applies the tiny [15,128,128] GEMM.

The PJRT executable (jit of shard_map over the 8 cores) is built once
and cached, replicated constants stay device-resident, per-batch shards
are uploaded asynchronously so packing overlaps the transfer, and the
result fetch is requested before blocking so its RTT hides under the
input transfer.  If an input activates more columns than the compiled
capacity CH*128, a numpy fallback computes the exact result.
"""

import numpy as np
from contextlib import ExitStack

import jax
from jax.sharding import Mesh, PartitionSpec, NamedSharding

import concourse.bass as bass
import concourse.mybir as mybir
import concourse.tile as tile
from concourse import bacc
from concourse.bass2jax import (
    _bass_exec_p,
    install_neuronx_cc_hook,
    partition_id_tensor,
)

try:
    from jax.experimental.shard_map import shard_map
except ImportError:
    from jax import shard_map

B, N, C, K = 8, 65536, 128, 15
KP_EXTENT = 1.0 * 1.2 / 2.5  # 0.48
CH = 96               # compiled capacity: chunks of 128 gathered columns
L = CH * 128          # 12288 gathered columns per core
XT = 2048             # x DMA tile free size
# block widths: full 2048-tiles then a 512-multiple remainder
_BLOCKS = []
_off = 0
while _off < L:
    _w = min(XT, L - _off)
    _BLOCKS.append((_off, _w))
    _off += _w

f32 = mybir.dt.float32
f16 = mybir.dt.float16


def _ap3(t, off_elems, d1, d2):
    """Build a 3-D access pattern [pdim, d1, d2] over tile ap `t`."""
    return bass.AP(t.tensor, t.offset + off_elems, [t.ap[0][:], list(d1), list(d2)])


def build_nc():
    nc = bacc.Bacc("TRN2", target_bir_lowering=False, debug=False, num_devices=B)

    i8 = mybir.dt.int8
    xq_d = nc.dram_tensor("xq", [C, L], i8, kind="ExternalInput")
    sg_d = nc.dram_tensor("sg", [128, CH], f16, kind="ExternalInput")
    pg_d = nc.dram_tensor("pg", [128, 3 * CH], f16, kind="ExternalInput")
    kb3_d = nc.dram_tensor("kb3", [128, 3 * K], f16, kind="ExternalInput")
    wsb_d = nc.dram_tensor("wsb", [C, K * 128], f16, kind="ExternalInput")
    eye16_d = nc.dram_tensor("eye16", [128, 128], f16, kind="ExternalInput")
    out_d = nc.dram_tensor("out", [1, 128], f32, kind="ExternalOutput")

    with tile.TileContext(nc) as tc, ExitStack() as ctx:
        consts = ctx.enter_context(tc.tile_pool(name="consts", bufs=1))
        tmp = ctx.enter_context(tc.tile_pool(name="tmp", bufs=3))
        xpool = ctx.enter_context(tc.tile_pool(name="xpool", bufs=3))
        xspool = ctx.enter_context(tc.tile_pool(name="xspool", bufs=6))
        ps_x = ctx.enter_context(tc.tile_pool(name="ps_x", bufs=4, space="PSUM"))
        ps_sm = ctx.enter_context(tc.tile_pool(name="ps_sm", bufs=2, space="PSUM"))
        ps_wf = ctx.enter_context(tc.tile_pool(name="ps_wf", bufs=1, space="PSUM"))
        fin = ctx.enter_context(tc.tile_pool(name="fin", bufs=1))

        eye16 = consts.tile([128, 128], f16)
        nc.sync.dma_start(eye16, eye16_d.ap())
        wsb = consts.tile([C, K * 128], f16)
        nc.sync.dma_start(wsb, wsb_d.ap())
        pg = consts.tile([128, 3 * CH], f16)
        nc.sync.dma_start(pg, pg_d.ap())
        sg = consts.tile([128, CH], f16)
        nc.sync.dma_start(sg, sg_d.ap())
        kb3 = consts.tile([128, 3 * K], f16)
        nc.sync.dma_start(kb3, kb3_d.ap())

        # aw[j, ch*K+k] = relu(1 - |p_active[ch*128+j] - kp[k]| / KP_EXTENT)
        awb = consts.tile([128, CH * K], f16)
        acc = None
        for d in range(3):
            dx = tmp.tile([128, CH * K], f16, tag="dx", name=f"dx{d}")
            dx3 = _ap3(dx, 0, [K, CH], [1, K])
            pb = _ap3(pg, d * CH, [1, CH], [0, K])
            kb = _ap3(kb3, d * K, [0, CH], [1, K])
            nc.vector.tensor_tensor(dx3, pb, kb, op=mybir.AluOpType.subtract)
            sx = tmp.tile([128, CH * K], f16, tag="sx", name=f"sx{d}")
            nc.vector.tensor_tensor(sx, dx, dx, op=mybir.AluOpType.mult)
            if acc is None:
                acc = sx
            else:
                a2 = tmp.tile([128, CH * K], f16, tag="acc", name=f"acc{d}")
                nc.vector.tensor_tensor(a2, acc, sx, op=mybir.AluOpType.add)
                acc = a2
        rt = tmp.tile([128, CH * K], f16, tag="rt")
        nc.scalar.sqrt(rt, acc)
        awr = tmp.tile([128, CH * K], f16, tag="awr")
        nc.scalar.activation(
            awr, rt, mybir.ActivationFunctionType.Relu,
            bias=1.0, scale=-1.0 / KP_EXTENT)
        # fold the int8 per-column dequant scale into aw
        sgb = _ap3(sg, 0, [1, CH], [0, K])
        awr3 = _ap3(awr, 0, [K, CH], [1, K])
        awb3 = _ap3(awb, 0, [K, CH], [1, K])
        nc.vector.tensor_tensor(awb3, awr3, sgb, op=mybir.AluOpType.mult)

        # wf[k,c] accumulated over all CH chunks of gathered columns
        wf = ps_wf.tile([K, 128], f32)
        for j, (off, w) in enumerate(_BLOCKS):
            xq = xpool.tile([128, w], i8, tag="xq", name=f"xq{j}")
            nc.sync.dma_start(xq, xq_d.ap()[:, off:off + w])
            # int8 -> f16 numeric conversion (dequant scale lives in awb)
            xt = xpool.tile([128, w], f16, tag="xt", name=f"xt{j}")
            nc.vector.tensor_copy(xt, xq)
            h0 = 0
            while h0 < w:
                hw = min(1024, w - h0)
                ps = ps_x.tile([128, hw], f16, tag="psx", name=f"psx{j}{h0}")
                for u in range(hw // 128):
                    nc.tensor.transpose(
                        ps[:, 128 * u:128 * (u + 1)],
                        xt[:, h0 + 128 * u:h0 + 128 * (u + 1)],
                        eye16)
                xs = xspool.tile([128, hw], f16, tag="xs", name=f"xs{j}{h0}")
                nc.vector.tensor_copy(xs, ps)
                for u in range(hw // 128):
                    ch = (off + h0) // 128 + u
                    nc.tensor.matmul(
                        wf, awb[:, K * ch:K * (ch + 1)],
                        xs[:, 128 * u:128 * (u + 1)],
                        start=(ch == 0), stop=(ch == CH - 1),
                        skip_group_check=True)
                h0 += hw

        # stage 2: out[o] = sum_k wf[k,:] @ W[k]
        wf_sb = fin.tile([K, 128], f16)
        nc.vector.tensor_copy(wf_sb, wf)
        wft_ps = ps_sm.tile([128, K], f16, tag="pt")
        nc.tensor.transpose(wft_ps, wf_sb, eye16[:K, :K])
        wft = fin.tile([128, K], f16)
        nc.vector.tensor_copy(wft, wft_ps)
        o_ps = ps_sm.tile([1, 128], f32, tag="pt")
        for k in range(K):
            nc.tensor.matmul(
                o_ps, wft[:, k:k + 1], wsb[:, 128 * k:128 * (k + 1)],
                start=(k == 0), stop=(k == K - 1), skip_group_check=True)
        o_sb = fin.tile([1, 128], f32)
        nc.vector.tensor_copy(o_sb, o_ps)
        nc.sync.dma_start(out_d.ap(), o_sb)

    nc.compile()
    return nc


def _host_aw(p, kp):
    """aw[b,n,k] = relu(1 - |p[b,n]-kp[k]|/KP_EXTENT), f32, exact."""
    d2 = ((p * p).sum(-1)[:, :, None] + (kp * kp).sum(-1)[None, None, :]
          - 2.0 * (p @ kp.T))
    aw = 1.0 - np.sqrt(np.maximum(d2, 0.0)) * np.float32(1.0 / KP_EXTENT)
    return np.maximum(aw, 0.0, out=aw)


def _active_mask(p, kp):
    """n is active iff min_k ||p[n]-kp[k]||^2 < KP_EXTENT^2 (no sqrt)."""
    d2 = ((p * p).sum(-1)[:, :, None] + (kp * kp).sum(-1)[None, None, :]
          - 2.0 * (p @ kp.T))
    return d2.min(axis=2) < np.float32(KP_EXTENT * KP_EXTENT)


def _pack_batch(pb, xb, idx):
    """Pack one batch's active columns -> (xq, sg, pg) arrays [128, ...]."""
    m = idx.size
    xq = np.zeros((128, L), np.int8)
    s = np.zeros(0, np.float32)
    if m:
        xa = xb[:, idx]                                   # [128, m] f32
        s = np.abs(xa).max(axis=0) * np.float32(1 / 127)  # per-column scale
        np.maximum(s, np.float32(1e-30), out=s)
        xq[:, :m] = np.rint(xa / s).astype(np.int8)
    # sg[j, ch] = s of point ch*128+j (0 for padding -> aw*s = 0)
    s_pad = np.zeros(L, np.float32)
    s_pad[:m] = s
    sg = np.ascontiguousarray(s_pad.reshape(CH, 128).T).astype(np.float16)
    p_pad = np.zeros((L, 3), np.float32)
    p_pad[:m] = pb[idx]
    # pg[j, d*CH+ch] = p_active[ch*128+j, d]
    pg = np.ascontiguousarray(
        p_pad.reshape(CH, 128, 3).transpose(1, 2, 0).reshape(128, 3 * CH)
    ).astype(np.float16)
    return {"xq": xq, "sg": sg, "pg": pg}


def pack_inputs(p, x, weights, kernel_points):
    """Gather active columns; build concat-ready [B*128, ...] arrays.

    Returns None if any batch activates more than L columns (caller
    falls back to the exact numpy path)."""
    p = np.asarray(p, np.float32)
    x = np.asarray(x, np.float32)
    kp = np.asarray(kernel_points, np.float32)

    act = _active_mask(p, kp)
    out = {"xq": np.empty((B * 128, L), np.int8),
           "sg": np.empty((B * 128, CH), np.float16),
           "pg": np.empty((B * 128, 3 * CH), np.float16)}
    for b in range(B):
        idx = np.flatnonzero(act[b])
        if idx.size > L:
            return None
        for name, arr in _pack_batch(p[b], x[b], idx).items():
            out[name][b * 128:(b + 1) * 128] = arr
    return out


def pack_consts(weights, kernel_points):
    w = np.asarray(weights, np.float32)
    kp = np.asarray(kernel_points, np.float32)
    wsb = np.ascontiguousarray(
        w.transpose(1, 0, 2).reshape(C, K * 128)).astype(np.float16)
    eye16 = np.eye(128, dtype=np.float16)
    # kb3[j, d*K+k] = kp[k, d], rows replicated
    kb3 = np.ascontiguousarray(
        np.broadcast_to(kp.T.reshape(1, 3 * K), (128, 3 * K))).astype(np.float16)

    def rep(a):
        return np.ascontiguousarray(
            np.broadcast_to(a[None], (B, *a.shape))).reshape(B * a.shape[0],
                                                             *a.shape[1:])
    return {"wsb": rep(wsb), "eye16": rep(eye16), "kb3": rep(kb3)}


class Runner:
    """Persistent jit of shard_map(bass_exec) over the 8 cores."""

    def __init__(self):
        install_neuronx_cc_hook()
        self.nc = nc = build_nc()
        pname = nc.partition_id_tensor.name if nc.partition_id_tensor else None
        in_names, out_names, out_avals = [], [], []
        for alloc in nc.m.functions[0].allocations:
            if not isinstance(alloc, mybir.MemoryLocationSet):
                continue
            name = alloc.memorylocations[0].name
            if alloc.kind == "ExternalInput":
                if name != pname:
                    in_names.append(name)
            elif alloc.kind == "ExternalOutput":
                out_names.append(name)
                out_avals.append(jax.core.ShapedArray(
                    tuple(alloc.tensor_shape), mybir.dt.np(alloc.dtype)))
        self.in_names, self.out_names, self.out_avals = in_names, out_names, out_avals
        all_in = list(in_names) + list(out_names)
        if pname is not None:
            all_in.append(pname)
        n_params, n_outs = len(in_names), len(out_names)
        donate = tuple(range(n_params, n_params + n_outs))

        def _body(*args):
            operands = list(args)
            if pname is not None:
                operands.append(partition_id_tensor())
            return tuple(_bass_exec_p.bind(
                *operands,
                out_avals=tuple(out_avals),
                in_names=tuple(all_in),
                out_names=tuple(out_names),
                lowering_input_output_aliases=(),
                sim_require_finite=True,
                sim_require_nnan=True,
                nc=nc,
            ))

        devices = jax.devices()[:B]
        self.mesh = Mesh(np.asarray(devices), ("core",))
        self.sharding = NamedSharding(self.mesh, PartitionSpec("core"))
        in_specs = (PartitionSpec("core"),) * (n_params + n_outs)
        out_specs = (PartitionSpec("core"),) * n_outs
        self.fn = jax.jit(
            shard_map(_body, mesh=self.mesh, in_specs=in_specs,
                      out_specs=out_specs, check_rep=False),
            donate_argnums=donate, keep_unused=True)
        self._const_key = None
        self._const_dev = None

    def put_consts(self, weights, kernel_points):
        """Device-resident replicated constants, re-uploaded only when
        the weights / kernel points actually change."""
        ids = (id(weights), id(kernel_points))
        if ids == getattr(self, "_const_ids", None):  # same arrays, fast path
            return self._const_dev
        w = np.asarray(weights)
        kp = np.asarray(kernel_points)
        key = hash((w.tobytes(), kp.tobytes()))
        if key != self._const_key:
            consts = pack_consts(w, kp)
            self._const_dev = {
                k: jax.device_put(v, self.sharding) for k, v in consts.items()}
            self._const_key = key
        self._const_ids = ids
        return self._const_dev

    def run(self, packed, const_dev):
        args = []
        for name in self.in_names:
            args.append(packed[name] if name in packed else const_dev[name])
        zeros = [np.zeros((B * a.shape[0], *a.shape[1:]), a.dtype)
                 for a in self.out_avals]
        outs = self.fn(*args, *zeros)
        # request the (tiny) result right away so the D2H round trip
        # overlaps the input transfer + execution instead of following it
        outs[0].copy_to_host_async()
        out = np.asarray(outs[0]).reshape(B, *self.out_avals[0].shape)
        return out.reshape(B, -1)


_RUNNER = None


def _get_runner():
    global _RUNNER
    if _RUNNER is None:
        _RUNNER = Runner()
    return _RUNNER


def _numpy_fallback(p, x, weights, kernel_points):
    aw = _host_aw(np.asarray(p, np.float32), np.asarray(kernel_points, np.float32))
    wf = np.einsum('bnk,bcn->bkc', aw, np.asarray(x, np.float32))
    return np.einsum('bkc,kco->bo', wf, np.asarray(weights, np.float32))


_GSHAPES = {"xq": (B * 128, L), "sg": (B * 128, CH),
            "pg": (B * 128, 3 * CH)}


def kernel(p, x, weights, kernel_points):
    p_ = np.asarray(p, np.float32)
    x_ = np.asarray(x, np.float32)
    kp_ = np.asarray(kernel_points, np.float32)

    try:
        r = _get_runner()
        const_dev = r.put_consts(weights, kernel_points)
        act = _active_mask(p_, kp_)
        idxs = [np.flatnonzero(act[b]) for b in range(B)]
        if max(i.size for i in idxs) > L:  # beyond compiled capacity
            return _numpy_fallback(p, x, weights, kernel_points).astype(np.float32)

        # pack batch-by-batch, shipping each core's shard asynchronously so
        # the host packing overlaps the (slow) host->device transfer
        devices = list(r.mesh.devices.flat)
        shards = {name: [] for name in _GSHAPES}
        for b in range(B):
            batch = _pack_batch(p_[b], x_[b], idxs[b])
            for name in shards:
                shards[name].append(jax.device_put(batch[name], devices[b]))
        packed = {name: jax.make_array_from_single_device_arrays(
            _GSHAPES[name], r.sharding, shards[name]) for name in shards}
        return r.run(packed, const_dev).astype(np.float32)
    except Exception as exc:  # device/tunnel failure: stay correct, just slow
        import sys
        print(f"kernel: device path failed ({type(exc).__name__}: {exc}); "
              f"using exact numpy fallback", file=sys.stderr)
        return _numpy_fallback(p, x, weights, kernel_points).astype(np.float32)


# revision 31
# speedup vs baseline: 1.1296x; 1.0328x over previous
"""KPConv aggregate layer on 8 trn2 NeuronCores.

Math (per batch b):
    sq_d[n,k]  = ||p[n] - kp[k]||^2
    aw[n,k]    = relu(1 - sqrt(sq_d)/KP_EXTENT)
    wf[k,c]    = sum_n aw[n,k] * x[c,n]
    out[o]     = sum_{k,c} wf[k,c] * W[k,c,o]

Sharding: data-parallel over B=8 across the 8 cores (batch b -> core b).

aw has a radius cutoff, so only columns n with min_k ||p[n]-kp[k]|| <
KP_EXTENT contribute (~17.5% of N on N(0,1) points).  The host gathers
the active columns of x and their point coords and ships only those —
everything else is exact zeros.  The dominant cost is the axon tunnel
(~75 MB/s aggregate, ~100 ms RTT), so x is shipped as int8 with a
per-column max scale; the device converts int8->fp16 and the dequant
scale is folded into aw (recomputed on device from the gathered
coords, then multiplied by the shipped scale vector).  The device
kernel PE-transposes the x tiles and accumulates wf with 15-wide
stationary matmuls into PSUM, then applies the tiny [15,128,128] GEMM.

The PJRT executable (jit of shard_map over the 8 cores) is built once
and cached, replicated constants stay device-resident, per-batch shards
are uploaded asynchronously so packing overlaps the transfer, and the
result fetch is requested before blocking so its RTT hides under the
input transfer.  If an input activates more columns than the compiled
capacity CH*128, a numpy fallback computes the exact result.
"""

import numpy as np
from contextlib import ExitStack

import jax
from jax.sharding import Mesh, PartitionSpec, NamedSharding

import concourse.bass as bass
import concourse.mybir as mybir
import concourse.tile as tile
from concourse import bacc
from concourse.bass2jax import (
    _bass_exec_p,
    install_neuronx_cc_hook,
    partition_id_tensor,
)

try:
    from jax.experimental.shard_map import shard_map
except ImportError:
    from jax import shard_map

B, N, C, K = 8, 65536, 128, 15
KP_EXTENT = 1.0 * 1.2 / 2.5  # 0.48
CH = 96               # compiled capacity: chunks of 128 gathered columns
L = CH * 128          # 12288 gathered columns per core
XT = 2048             # x DMA tile free size
# block widths: full 2048-tiles then a 512-multiple remainder
_BLOCKS = []
_off = 0
while _off < L:
    _w = min(XT, L - _off)
    _BLOCKS.append((_off, _w))
    _off += _w

f32 = mybir.dt.float32
f16 = mybir.dt.float16


def _ap3(t, off_elems, d1, d2):
    """Build a 3-D access pattern [pdim, d1, d2] over tile ap `t`."""
    return bass.AP(t.tensor, t.offset + off_elems, [t.ap[0][:], list(d1), list(d2)])


def build_nc():
    nc = bacc.Bacc("TRN2", target_bir_lowering=False, debug=False, num_devices=B)

    i8 = mybir.dt.int8
    xq_d = nc.dram_tensor("xq", [C, L], i8, kind="ExternalInput")
    sg_d = nc.dram_tensor("sg", [128, CH], f16, kind="ExternalInput")
    pg_d = nc.dram_tensor("pg", [128, 3 * CH], f16, kind="ExternalInput")
    kb3_d = nc.dram_tensor("kb3", [128, 3 * K], f16, kind="ExternalInput")
    wsb_d = nc.dram_tensor("wsb", [C, K * 128], f16, kind="ExternalInput")
    eye16_d = nc.dram_tensor("eye16", [128, 128], f16, kind="ExternalInput")
    out_d = nc.dram_tensor("out", [1, 128], f32, kind="ExternalOutput")

    with tile.TileContext(nc) as tc, ExitStack() as ctx:
        consts = ctx.enter_context(tc.tile_pool(name="consts", bufs=1))
        tmp = ctx.enter_context(tc.tile_pool(name="tmp", bufs=3))
        xpool = ctx.enter_context(tc.tile_pool(name="xpool", bufs=3))
        xspool = ctx.enter_context(tc.tile_pool(name="xspool", bufs=6))
        ps_x = ctx.enter_context(tc.tile_pool(name="ps_x", bufs=4, space="PSUM"))
        ps_sm = ctx.enter_context(tc.tile_pool(name="ps_sm", bufs=2, space="PSUM"))
        ps_wf = ctx.enter_context(tc.tile_pool(name="ps_wf", bufs=1, space="PSUM"))
        fin = ctx.enter_context(tc.tile_pool(name="fin", bufs=1))

        eye16 = consts.tile([128, 128], f16)
        nc.sync.dma_start(eye16, eye16_d.ap())
        wsb = consts.tile([C, K * 128], f16)
        nc.sync.dma_start(wsb, wsb_d.ap())
        pg = consts.tile([128, 3 * CH], f16)
        nc.sync.dma_start(pg, pg_d.ap())
        sg = consts.tile([128, CH], f16)
        nc.sync.dma_start(sg, sg_d.ap())
        kb3 = consts.tile([128, 3 * K], f16)
        nc.sync.dma_start(kb3, kb3_d.ap())

        # aw[j, ch*K+k] = relu(1 - |p_active[ch*128+j] - kp[k]| / KP_EXTENT)
        awb = consts.tile([128, CH * K], f16)
        acc = None
        for d in range(3):
            dx = tmp.tile([128, CH * K], f16, tag="dx", name=f"dx{d}")
            dx3 = _ap3(dx, 0, [K, CH], [1, K])
            pb = _ap3(pg, d * CH, [1, CH], [0, K])
            kb = _ap3(kb3, d * K, [0, CH], [1, K])
            nc.vector.tensor_tensor(dx3, pb, kb, op=mybir.AluOpType.subtract)
            sx = tmp.tile([128, CH * K], f16, tag="sx", name=f"sx{d}")
            nc.vector.tensor_tensor(sx, dx, dx, op=mybir.AluOpType.mult)
            if acc is None:
                acc = sx
            else:
                a2 = tmp.tile([128, CH * K], f16, tag="acc", name=f"acc{d}")
                nc.vector.tensor_tensor(a2, acc, sx, op=mybir.AluOpType.add)
                acc = a2
        rt = tmp.tile([128, CH * K], f16, tag="rt")
        nc.scalar.sqrt(rt, acc)
        awr = tmp.tile([128, CH * K], f16, tag="awr")
        nc.scalar.activation(
            awr, rt, mybir.ActivationFunctionType.Relu,
            bias=1.0, scale=-1.0 / KP_EXTENT)
        # fold the int8 per-column dequant scale into aw
        sgb = _ap3(sg, 0, [1, CH], [0, K])
        awr3 = _ap3(awr, 0, [K, CH], [1, K])
        awb3 = _ap3(awb, 0, [K, CH], [1, K])
        nc.vector.tensor_tensor(awb3, awr3, sgb, op=mybir.AluOpType.mult)

        # wf[k,c] accumulated over all CH chunks of gathered columns
        wf = ps_wf.tile([K, 128], f32)
        for j, (off, w) in enumerate(_BLOCKS):
            xq = xpool.tile([128, w], i8, tag="xq", name=f"xq{j}")
            nc.sync.dma_start(xq, xq_d.ap()[:, off:off + w])
            # int8 -> f16 numeric conversion (dequant scale lives in awb)
            xt = xpool.tile([128, w], f16, tag="xt", name=f"xt{j}")
            nc.vector.tensor_copy(xt, xq)
            h0 = 0
            while h0 < w:
                hw = min(1024, w - h0)
                ps = ps_x.tile([128, hw], f16, tag="psx", name=f"psx{j}{h0}")
                for u in range(hw // 128):
                    nc.tensor.transpose(
                        ps[:, 128 * u:128 * (u + 1)],
                        xt[:, h0 + 128 * u:h0 + 128 * (u + 1)],
                        eye16)
                xs = xspool.tile([128, hw], f16, tag="xs", name=f"xs{j}{h0}")
                nc.vector.tensor_copy(xs, ps)
                for u in range(hw // 128):
                    ch = (off + h0) // 128 + u
                    nc.tensor.matmul(
                        wf, awb[:, K * ch:K * (ch + 1)],
                        xs[:, 128 * u:128 * (u + 1)],
                        start=(ch == 0), stop=(ch == CH - 1),
                        skip_group_check=True)
                h0 += hw

        # stage 2: out[o] = sum_k wf[k,:] @ W[k]
        wf_sb = fin.tile([K, 128], f16)
        nc.vector.tensor_copy(wf_sb, wf)
        wft_ps = ps_sm.tile([128, K], f16, tag="pt")
        nc.tensor.transpose(wft_ps, wf_sb, eye16[:K, :K])
        wft = fin.tile([128, K], f16)
        nc.vector.tensor_copy(wft, wft_ps)
        o_ps = ps_sm.tile([1, 128], f32, tag="pt")
        for k in range(K):
            nc.tensor.matmul(
                o_ps, wft[:, k:k + 1], wsb[:, 128 * k:128 * (k + 1)],
                start=(k == 0), stop=(k == K - 1), skip_group_check=True)
        o_sb = fin.tile([1, 128], f32)
        nc.vector.tensor_copy(o_sb, o_ps)
        nc.sync.dma_start(out_d.ap(), o_sb)

    nc.compile()
    return nc


def _host_aw(p, kp):
    """aw[b,n,k] = relu(1 - |p[b,n]-kp[k]|/KP_EXTENT), f32, exact."""
    d2 = ((p * p).sum(-1)[:, :, None] + (kp * kp).sum(-1)[None, None, :]
          - 2.0 * (p @ kp.T))
    aw = 1.0 - np.sqrt(np.maximum(d2, 0.0)) * np.float32(1.0 / KP_EXTENT)
    return np.maximum(aw, 0.0, out=aw)


def _active_mask(p, kp):
    """n is active iff min_k ||p[n]-kp[k]||^2 < KP_EXTENT^2 (no sqrt)."""
    d2 = ((p * p).sum(-1)[:, :, None] + (kp * kp).sum(-1)[None, None, :]
          - 2.0 * (p @ kp.T))
    return d2.min(axis=2) < np.float32(KP_EXTENT * KP_EXTENT)


def _pack_batch(pb, xb, idx):
    """Pack one batch's active columns -> (xq, sg, pg) arrays [128, ...]."""
    m = idx.size
    xq = np.zeros((128, L), np.int8)
    s = np.zeros(0, np.float32)
    if m:
        xa = xb[:, idx]                                   # [128, m] f32
        s = np.abs(xa).max(axis=0) * np.float32(1 / 127)  # per-column scale
        np.maximum(s, np.float32(1e-30), out=s)
        xq[:, :m] = np.rint(xa / s).astype(np.int8)
    # sg[j, ch] = s of point ch*128+j (0 for padding -> aw*s = 0)
    s_pad = np.zeros(L, np.float32)
    s_pad[:m] = s
    sg = np.ascontiguousarray(s_pad.reshape(CH, 128).T).astype(np.float16)
    p_pad = np.zeros((L, 3), np.float32)
    p_pad[:m] = pb[idx]
    # pg[j, d*CH+ch] = p_active[ch*128+j, d]
    pg = np.ascontiguousarray(
        p_pad.reshape(CH, 128, 3).transpose(1, 2, 0).reshape(128, 3 * CH)
    ).astype(np.float16)
    return {"xq": xq, "sg": sg, "pg": pg}


def pack_inputs(p, x, weights, kernel_points):
    """Gather active columns; build concat-ready [B*128, ...] arrays.

    Returns None if any batch activates more than L columns (caller
    falls back to the exact numpy path)."""
    p = np.asarray(p, np.float32)
    x = np.asarray(x, np.float32)
    kp = np.asarray(kernel_points, np.float32)

    act = _active_mask(p, kp)
    out = {"xq": np.empty((B * 128, L), np.int8),
           "sg": np.empty((B * 128, CH), np.float16),
           "pg": np.empty((B * 128, 3 * CH), np.float16)}
    for b in range(B):
        idx = np.flatnonzero(act[b])
        if idx.size > L:
            return None
        for name, arr in _pack_batch(p[b], x[b], idx).items():
            out[name][b * 128:(b + 1) * 128] = arr
    return out


def pack_consts(weights, kernel_points):
    w = np.asarray(weights, np.float32)
    kp = np.asarray(kernel_points, np.float32)
    wsb = np.ascontiguousarray(
        w.transpose(1, 0, 2).reshape(C, K * 128)).astype(np.float16)
    eye16 = np.eye(128, dtype=np.float16)
    # kb3[j, d*K+k] = kp[k, d], rows replicated
    kb3 = np.ascontiguousarray(
        np.broadcast_to(kp.T.reshape(1, 3 * K), (128, 3 * K))).astype(np.float16)

    def rep(a):
        return np.ascontiguousarray(
            np.broadcast_to(a[None], (B, *a.shape))).reshape(B * a.shape[0],
                                                             *a.shape[1:])
    return {"wsb": rep(wsb), "eye16": rep(eye16), "kb3": rep(kb3)}


class Runner:
    """Persistent jit of shard_map(bass_exec) over the 8 cores."""

    def __init__(self):
        install_neuronx_cc_hook()
        self.nc = nc = build_nc()
        pname = nc.partition_id_tensor.name if nc.partition_id_tensor else None
        in_names, out_names, out_avals = [], [], []
        for alloc in nc.m.functions[0].allocations:
            if not isinstance(alloc, mybir.MemoryLocationSet):
                continue
            name = alloc.memorylocations[0].name
            if alloc.kind == "ExternalInput":
                if name != pname:
                    in_names.append(name)
            elif alloc.kind == "ExternalOutput":
                out_names.append(name)
                out_avals.append(jax.core.ShapedArray(
                    tuple(alloc.tensor_shape), mybir.dt.np(alloc.dtype)))
        self.in_names, self.out_names, self.out_avals = in_names, out_names, out_avals
        all_in = list(in_names) + list(out_names)
        if pname is not None:
            all_in.append(pname)
        n_params, n_outs = len(in_names), len(out_names)
        donate = tuple(range(n_params, n_params + n_outs))

        def _body(*args):
            operands = list(args)
            if pname is not None:
                operands.append(partition_id_tensor())
            return tuple(_bass_exec_p.bind(
                *operands,
                out_avals=tuple(out_avals),
                in_names=tuple(all_in),
                out_names=tuple(out_names),
                lowering_input_output_aliases=(),
                sim_require_finite=True,
                sim_require_nnan=True,
                nc=nc,
            ))

        devices = jax.devices()[:B]
        self.mesh = Mesh(np.asarray(devices), ("core",))
        self.sharding = NamedSharding(self.mesh, PartitionSpec("core"))
        in_specs = (PartitionSpec("core"),) * (n_params + n_outs)
        out_specs = (PartitionSpec("core"),) * n_outs
        self.fn = jax.jit(
            shard_map(_body, mesh=self.mesh, in_specs=in_specs,
                      out_specs=out_specs, check_rep=False),
            donate_argnums=donate, keep_unused=True)
        self._const_key = None
        self._const_dev = None

    def put_consts(self, weights, kernel_points):
        """Device-resident replicated constants, re-uploaded only when
        the weights / kernel points actually change."""
        ids = (id(weights), id(kernel_points))
        if ids == getattr(self, "_const_ids", None):  # same arrays, fast path
            return self._const_dev
        w = np.asarray(weights)
        kp = np.asarray(kernel_points)
        key = hash((w.tobytes(), kp.tobytes()))
        if key != self._const_key:
            consts = pack_consts(w, kp)
            self._const_dev = {
                k: jax.device_put(v, self.sharding) for k, v in consts.items()}
            self._const_key = key
        self._const_ids = ids
        return self._const_dev

    def run(self, packed, const_dev):
        args = []
        for name in self.in_names:
            args.append(packed[name] if name in packed else const_dev[name])
        zeros = [np.zeros((B * a.shape[0], *a.shape[1:]), a.dtype)
                 for a in self.out_avals]
        outs = self.fn(*args, *zeros)
        # request the (tiny) result right away so the D2H round trip
        # overlaps the input transfer + execution instead of following it
        outs[0].copy_to_host_async()
        out = np.asarray(outs[0]).reshape(B, *self.out_avals[0].shape)
        return out.reshape(B, -1)


_RUNNER = None


def _get_runner():
    global _RUNNER
    if _RUNNER is None:
        _RUNNER = Runner()
    return _RUNNER


def _numpy_fallback(p, x, weights, kernel_points):
    aw = _host_aw(np.asarray(p, np.float32), np.asarray(kernel_points, np.float32))
    wf = np.einsum('bnk,bcn->bkc', aw, np.asarray(x, np.float32))
    return np.einsum('bkc,kco->bo', wf, np.asarray(weights, np.float32))


_GSHAPES = {"xq": (B * 128, L), "sg": (B * 128, CH),
            "pg": (B * 128, 3 * CH)}


def kernel(p, x, weights, kernel_points):
    p_ = np.asarray(p, np.float32)
    x_ = np.asarray(x, np.float32)
    kp_ = np.asarray(kernel_points, np.float32)

    try:
        r = _get_runner()
        const_dev = r.put_consts(weights, kernel_points)
        act = _active_mask(p_, kp_)
        idxs = [np.flatnonzero(act[b]) for b in range(B)]
        if max(i.size for i in idxs) > L:  # beyond compiled capacity
            return _numpy_fallback(p, x, weights, kernel_points).astype(np.float32)

        # pack batches in worker threads (numpy releases the GIL on the big
        # ufuncs) and ship each core's shard asynchronously as soon as it is
        # ready, so packing overlaps the (slow) host->device transfer
        from concurrent.futures import ThreadPoolExecutor
        devices = list(r.mesh.devices.flat)
        shards = {name: [None] * B for name in _GSHAPES}
        with ThreadPoolExecutor(4) as ex:
            futs = [ex.submit(_pack_batch, p_[b], x_[b], idxs[b])
                    for b in range(B)]
            for b, fut in enumerate(futs):
                batch = fut.result()
                for name in shards:
                    shards[name][b] = jax.device_put(batch[name], devices[b])
        packed = {name: jax.make_array_from_single_device_arrays(
            _GSHAPES[name], r.sharding, shards[name]) for name in shards}
        return r.run(packed, const_dev).astype(np.float32)
    except Exception as exc:  # device/tunnel failure: stay correct, just slow
        import sys
        print(f"kernel: device path failed ({type(exc).__name__}: {exc}); "
              f"using exact numpy fallback", file=sys.stderr)
        return _numpy_fallback(p, x, weights, kernel_points).astype(np.float32)
